# revision 15
# baseline (speedup 1.0000x reference)
"""ObjectAttentionBlock (dense transformer) Trainium2 Bass kernel.

Full-input contract: kernel(**inputs) takes the unsharded inputs and
returns the unsharded output. Internally the batch dimension (N=8) is
data-parallel across the 8 NeuronCores: core n processes batch element n.

Per-core computation (channels-first layout [C, pixels] everywhere):
  q  = relu(s1*(W1 @ relu(s0*(W0 @ x) + b0)) + b1)          [C, HW]
  key= relu(s3*(W3 @ relu(s2*(W2 @ p) + b2)) + b3)          [C, K]
  val= relu(s5*(W5 @ relu(s4*(W4 @ p) + b4)) + b5)          [C, K]
  sim= softmax(q^T key / sqrt(C), axis=K)                    [HW, K]
  ctx= (sim @ val^T)^T                                       [C, HW]
  out= relu(s6*(W6 @ ctx) + b6)                              [C, HW]

All matmuls run as float32r (TF32-like single-pass mode, 4x the fp32
matmul rate on the PE).

The attention part is computed k-major without any transposes of the
probabilities: simT[k, pix] = key^T q accumulates directly on the PE;
exp() is applied without max-subtraction (alpha*sim is in [0, ~5]
because q and key are post-ReLU — no overflow possible); the softmax
row-sum is obtained with a ones-matrix matmul that replicates the
per-pixel sum across all 128 partitions, and the normalization is fused
into the ctx PSUM->SBUF eviction as a tensor_tensor multiply with the
reciprocal.
"""

import numpy as np

import concourse.bass as bass
import concourse.mybir as mybir
import concourse.tile as tile
from concourse import bacc, bass_utils

N = 8
C = 512
K = 256
H = 128
W_IMG = 128
HW = H * W_IMG
P = 128          # partition width
CT = C // P      # 4 channel tiles
KT = K // P      # 2 region tiles
T = 512          # pixel tile (matmul moving dim / one PSUM bank of fp32)
NT = HW // T     # 32 pixel tiles
ALPHA = 1.0 / float(np.sqrt(C))

f32 = mybir.dt.float32
f32r = mybir.dt.float32r
AF = mybir.ActivationFunctionType


def build_module(n_tiles=NT):
    """Build and compile the per-core Bass module (SPMD: same on all cores)."""
    nc = bacc.Bacc("TRN2", target_bir_lowering=False, debug=False)
    xin = nc.dram_tensor("xin", [C, HW], f32, kind="ExternalInput").ap()
    pin = nc.dram_tensor("pin", [C, K], f32, kind="ExternalInput").ap()
    wt = nc.dram_tensor("wt", [7, C, C], f32, kind="ExternalInput").ap()
    sbc_d = nc.dram_tensor("sbc", [P, 28], f32, kind="ExternalInput").ap()
    idn_d = nc.dram_tensor("idn", [P, P], f32, kind="ExternalInput").ap()
    ones_d = nc.dram_tensor("ones", [P, P], f32, kind="ExternalInput").ap()
    out_d = nc.dram_tensor("out", [C, HW], f32, kind="ExternalOutput").ap()

    with tile.TileContext(nc) as tc:
        with (
            tc.tile_pool(name="const", bufs=1) as cpool,
            tc.tile_pool(name="loop", bufs=2) as lpool,
            tc.tile_pool(name="psc", bufs=4, space="PSUM") as psp,
            tc.tile_pool(name="psa", bufs=4, space="PSUM") as psa,
        ):
            # ---- constants ----
            sbc = cpool.tile([P, 28], f32, name="sbc_t")
            nc.sync.dma_start(sbc[:], sbc_d[:])
            idn = cpool.tile([P, P], f32, name="idn_t")
            nc.sync.dma_start(idn[:].bitcast(f32r), idn_d[:].bitcast(f32r))
            ones = cpool.tile([P, P], f32, name="ones_t")
            nc.sync.dma_start(ones[:].bitcast(f32r), ones_d[:].bitcast(f32r))
            w = [
                [cpool.tile([P, C], f32, name=f"w{i}_{c}") for c in range(CT)]
                for i in range(7)
            ]
            p_t = [cpool.tile([P, K], f32, name=f"p{c}") for c in range(CT)]
            for c in range(CT):
                nc.gpsimd.dma_start(
                    p_t[c][:].bitcast(f32r),
                    pin[c * P : (c + 1) * P, :].bitcast(f32r),
                )
            for c in range(CT):
                nc.sync.dma_start(
                    w[2][c][:].bitcast(f32r),
                    wt[2, c * P : (c + 1) * P, :].bitcast(f32r),
                )
            xt0 = [
                lpool.tile([P, T], f32, name=f"xt{c}", tag=f"xt{c}")
                for c in range(CT)
            ]
            for c in range(CT):
                nc.sync.dma_start(
                    xt0[c][:].bitcast(f32r),
                    xin[c * P : (c + 1) * P, 0:T].bitcast(f32r),
                )
            for i in (4, 0, 3, 5, 1, 6):
                for c in range(CT):
                    nc.gpsimd.dma_start(
                        w[i][c][:].bitcast(f32r),
                        wt[i, c * P : (c + 1) * P, :].bitcast(f32r),
                    )

            def bias_ap(i, o):
                return sbc[:, i * 4 + o : i * 4 + o + 1]

            def conv(inp, wi, outt, ncols, out_f32r=True, relu_engine="act"):
                """outt[o] = relu(W[wi]@inp + b); inp/outt: CT tiles [P, ncols]."""
                for o in range(CT):
                    ps = psp.tile([P, ncols], f32, name=f"ps_{wi}_{o}", tag="cps")
                    for c in range(CT):
                        nc.tensor.matmul(
                            ps[:],
                            w[wi][c][:, o * P : (o + 1) * P].bitcast(f32r),
                            inp[c][:].bitcast(f32r),
                            start=(c == 0),
                            stop=(c == CT - 1),
                        )
                    dst = outt[o][:].bitcast(f32r) if out_f32r else outt[o][:]
                    if relu_engine == "act":
                        nc.scalar.activation(dst, ps[:], AF.Relu, bias=bias_ap(wi, o))
                    else:
                        nc.vector.tensor_scalar(
                            out=dst,
                            in0=ps[:],
                            scalar1=bias_ap(wi, o),
                            scalar2=0.0,
                            op0=mybir.AluOpType.add,
                            op1=mybir.AluOpType.max,
                        )

            # ---- setup: key / value from proxy ----
            key = [cpool.tile([P, K], f32, name=f"key{c}") for c in range(CT)]
            valT = [cpool.tile([P, C], f32, name=f"valT{k}") for k in range(KT)]
            with tc.tile_pool(name="setup", bufs=1) as spool:
                k1 = [spool.tile([P, K], f32, name=f"k1_{c}") for c in range(CT)]
                conv(p_t, 2, k1, K)
                conv(k1, 3, key, K)
                v1 = [spool.tile([P, K], f32, name=f"v1_{c}") for c in range(CT)]
                conv(p_t, 4, v1, K)
                val = [spool.tile([P, K], f32, name=f"val{c}") for c in range(CT)]
                conv(v1, 5, val, K)
                for k in range(KT):
                    pt = psa.tile([P, C], f32, name=f"ptv{k}", tag="aps")
                    for c in range(CT):
                        nc.tensor.matmul(
                            pt[:, c * P : (c + 1) * P].bitcast(f32r),
                            val[c][:, k * P : (k + 1) * P].bitcast(f32r),
                            idn[:].bitcast(f32r),
                            is_transpose=True,
                        )
                    nc.vector.tensor_copy(valT[k][:].bitcast(f32r), pt[:])

            # ---- main pipeline over pixel tiles ----
            def stage_a(t, xt=None):
                if xt is None:
                    xt = [
                        lpool.tile([P, T], f32, name=f"xt{c}", tag=f"xt{c}")
                        for c in range(CT)
                    ]
                    for c in range(CT):
                        nc.sync.dma_start(
                            xt[c][:].bitcast(f32r),
                            xin[c * P : (c + 1) * P, t * T : (t + 1) * T].bitcast(f32r),
                        )
                t1 = [lpool.tile([P, T], f32, name=f"t1_{c}", tag=f"t1{c}") for c in range(CT)]
                conv(xt, 0, t1, T)
                q = [lpool.tile([P, T], f32, name=f"q{c}", tag=f"q{c}") for c in range(CT)]
                conv(t1, 1, q, T)
                probT = [
                    lpool.tile([P, T], f32, name=f"pT{k}", tag=f"pT{k}")
                    for k in range(KT)
                ]
                for k in range(KT):
                    ps = psa.tile([P, T], f32, name=f"ps_simT{k}", tag="aps")
                    for c in range(CT):
                        nc.tensor.matmul(
                            ps[:],
                            key[c][:, k * P : (k + 1) * P].bitcast(f32r),
                            q[c][:].bitcast(f32r),
                            start=(c == 0),
                            stop=(c == CT - 1),
                        )
                    nc.scalar.activation(
                        probT[k][:].bitcast(f32r), ps[:], AF.Exp, scale=ALPHA
                    )
                return probT

            def stage_b(t, probT):
                ps_rs = psa.tile([P, T], f32, name="ps_rs", tag="aps")
                for k in range(KT):
                    nc.tensor.matmul(
                        ps_rs[:],
                        ones[:].bitcast(f32r),
                        probT[k][:].bitcast(f32r),
                        start=(k == 0),
                        stop=(k == KT - 1),
                    )
                rc = lpool.tile([P, T], f32, name="rc", tag="rc")
                nc.vector.reciprocal_approx_fast(out=rc[:], in_=ps_rs[:])
                ctx = [lpool.tile([P, T], f32, name=f"cx{c}", tag=f"cx{c}") for c in range(CT)]
                for c in range(CT):
                    ps = psa.tile([P, T], f32, name=f"ps_ctx{c}", tag="aps")
                    for k in range(KT):
                        nc.tensor.matmul(
                            ps[:],
                            valT[k][:, c * P : (c + 1) * P].bitcast(f32r),
                            probT[k][:].bitcast(f32r),
                            start=(k == 0),
                            stop=(k == KT - 1),
                        )
                    nc.vector.tensor_tensor(
                        out=ctx[c][:].bitcast(f32r),
                        in0=ps[:],
                        in1=rc[:],
                        op=mybir.AluOpType.mult,
                    )
                outt = [
                    lpool.tile([P, T], f32, name=f"ot{o}", tag=f"ot{o}") for o in range(CT)
                ]
                conv(ctx, 6, outt, T, out_f32r=False)
                for o in range(CT):
                    nc.sync.dma_start(
                        out_d[o * P : (o + 1) * P, t * T : (t + 1) * T], outt[o][:]
                    )

            prev = None
            for t in range(n_tiles):
                pT = stage_a(t, xt=xt0 if t == 0 else None)
                if prev is not None:
                    stage_b(prev[0], prev[1])
                prev = (t, pT)
            stage_b(prev[0], prev[1])

    nc.compile()
    return nc


def make_in_maps(x, proxy, W, s, b):
    # s > 0, so relu(s*(W@x)+b) == relu((diag(s)W)@x + b): fold s into W.
    w_eff = s[:, :, None].astype(np.float64) * W.astype(np.float64)
    wt = np.ascontiguousarray(w_eff.transpose(0, 2, 1)).astype(np.float32)
    sbc = np.ascontiguousarray(
        b.reshape(7, CT, P).transpose(2, 0, 1).reshape(P, 7 * CT)
    ).astype(np.float32)
    idn = np.eye(P, dtype=np.float32)
    ones = np.ones((P, P), dtype=np.float32)
    in_maps = []
    for n in range(N):
        in_maps.append(
            {
                "xin": np.ascontiguousarray(x[n].reshape(C, HW), dtype=np.float32),
                "pin": np.ascontiguousarray(proxy[n].reshape(C, K), dtype=np.float32),
                "wt": wt,
                "sbc": sbc,
                "idn": idn,
                "ones": ones,
            }
        )
    return in_maps


_CACHED = {}


def _get_module():
    if "nc" not in _CACHED:
        _CACHED["nc"] = build_module()
    return _CACHED["nc"]


def kernel(x, proxy, W, s, b):
    nc = _get_module()
    in_maps = make_in_maps(x, proxy, W, s, b)
    res = bass_utils.run_bass_kernel_spmd(nc, in_maps, core_ids=list(range(N)))
    out = np.stack([res.results[n]["out"].reshape(C, H, W_IMG) for n in range(N)])
    return out.astype(np.float32)


# revision 16
# speedup vs baseline: 1.2026x; 1.2026x over previous
"""ObjectAttentionBlock (dense transformer) Trainium2 Bass kernel.

Full-input contract: kernel(**inputs) takes the unsharded inputs and
returns the unsharded output. Internally the batch dimension (N=8) is
data-parallel across the 8 NeuronCores: core n processes batch element n.

Per-core computation (channels-first layout [C, pixels] everywhere):
  q  = relu(s1*(W1 @ relu(s0*(W0 @ x) + b0)) + b1)          [C, HW]
  key= relu(s3*(W3 @ relu(s2*(W2 @ p) + b2)) + b3)          [C, K]
  val= relu(s5*(W5 @ relu(s4*(W4 @ p) + b4)) + b5)          [C, K]
  sim= softmax(q^T key / sqrt(C), axis=K)                    [HW, K]
  ctx= (sim @ val^T)^T                                       [C, HW]
  out= relu(s6*(W6 @ ctx) + b6)                              [C, HW]

All matmuls run as float32r (TF32-like single-pass mode, 4x the fp32
matmul rate on the PE).

The attention part is computed k-major without any transposes of the
probabilities: simT[k, pix] = key^T q accumulates directly on the PE;
exp() is applied without max-subtraction (alpha*sim is in [0, ~5]
because q and key are post-ReLU — no overflow possible); the softmax
row-sum is obtained with a ones-matrix matmul that replicates the
per-pixel sum across all 128 partitions, and the normalization is fused
into the ctx PSUM->SBUF eviction as a tensor_tensor multiply with the
reciprocal.
"""

import numpy as np

import concourse.bass as bass
import concourse.mybir as mybir
import concourse.tile as tile
from concourse import bacc, bass_utils

N = 8
C = 512
K = 256
H = 128
W_IMG = 128
HW = H * W_IMG
P = 128          # partition width
CT = C // P      # 4 channel tiles
KT = K // P      # 2 region tiles
T = 512          # pixel tile (matmul moving dim / one PSUM bank of fp32)
NT = HW // T     # 32 pixel tiles
ALPHA = 1.0 / float(np.sqrt(C))

f32 = mybir.dt.float32
f32r = mybir.dt.float32r
AF = mybir.ActivationFunctionType


def build_module(n_tiles=NT):
    """Build and compile the per-core Bass module (SPMD: same on all cores)."""
    nc = bacc.Bacc("TRN2", target_bir_lowering=False, debug=False)
    xin = nc.dram_tensor("xin", [C, HW], f32, kind="ExternalInput").ap()
    pin = nc.dram_tensor("pin", [C, K], f32, kind="ExternalInput").ap()
    wt = nc.dram_tensor("wt", [7, C, C], f32, kind="ExternalInput").ap()
    sbc_d = nc.dram_tensor("sbc", [P, 28], f32, kind="ExternalInput").ap()
    idn_d = nc.dram_tensor("idn", [P, P], f32, kind="ExternalInput").ap()
    ones_d = nc.dram_tensor("ones", [P, P], f32, kind="ExternalInput").ap()
    out_d = nc.dram_tensor("out", [C, HW], f32, kind="ExternalOutput").ap()

    with tile.TileContext(nc) as tc:
        with (
            tc.tile_pool(name="const", bufs=1) as cpool,
            tc.tile_pool(name="loop", bufs=2) as lpool,
            tc.tile_pool(name="psc", bufs=4, space="PSUM") as psp,
            tc.tile_pool(name="psa", bufs=4, space="PSUM") as psa,
        ):
            # ---- constants ----
            sbc = cpool.tile([P, 28], f32, name="sbc_t")
            nc.gpsimd.dma_start(sbc[:], sbc_d[:])
            idn = cpool.tile([P, P], f32, name="idn_t")
            nc.gpsimd.dma_start(idn[:].bitcast(f32r), idn_d[:].bitcast(f32r))
            ones = cpool.tile([P, P], f32, name="ones_t")
            nc.gpsimd.dma_start(ones[:].bitcast(f32r), ones_d[:].bitcast(f32r))
            w = [
                [cpool.tile([P, C], f32, name=f"w{i}_{c}") for c in range(CT)]
                for i in range(7)
            ]
            p_t = [cpool.tile([P, K], f32, name=f"p{c}") for c in range(CT)]
            for c in range(CT):
                nc.gpsimd.dma_start(
                    p_t[c][:].bitcast(f32r),
                    pin[c * P : (c + 1) * P, :].bitcast(f32r),
                )
            xt0 = [
                lpool.tile([P, T], f32, name=f"xt{c}", tag=f"xt{c}")
                for c in range(CT)
            ]
            for c in range(CT):
                nc.sync.dma_start(
                    xt0[c][:].bitcast(f32r),
                    xin[c * P : (c + 1) * P, 0:T].bitcast(f32r),
                )
            for i in (2, 4, 0, 3, 5, 1, 6):
                for c in range(CT):
                    nc.gpsimd.dma_start(
                        w[i][c][:].bitcast(f32r),
                        wt[i, c * P : (c + 1) * P, :].bitcast(f32r),
                    )

            def bias_ap(i, o):
                return sbc[:, i * 4 + o : i * 4 + o + 1]

            def conv(inp, wi, outt, ncols, out_f32r=True, relu_engine="act"):
                """outt[o] = relu(W[wi]@inp + b); inp/outt: CT tiles [P, ncols]."""
                for o in range(CT):
                    ps = psp.tile([P, ncols], f32, name=f"ps_{wi}_{o}", tag="cps")
                    for c in range(CT):
                        nc.tensor.matmul(
                            ps[:],
                            w[wi][c][:, o * P : (o + 1) * P].bitcast(f32r),
                            inp[c][:].bitcast(f32r),
                            start=(c == 0),
                            stop=(c == CT - 1),
                        )
                    dst = outt[o][:].bitcast(f32r) if out_f32r else outt[o][:]
                    if relu_engine == "act":
                        nc.scalar.activation(dst, ps[:], AF.Relu, bias=bias_ap(wi, o))
                    else:
                        nc.vector.tensor_scalar(
                            out=dst,
                            in0=ps[:],
                            scalar1=bias_ap(wi, o),
                            scalar2=0.0,
                            op0=mybir.AluOpType.add,
                            op1=mybir.AluOpType.max,
                        )

            # ---- setup: key / value from proxy ----
            key = [cpool.tile([P, K], f32, name=f"key{c}") for c in range(CT)]
            valT = [cpool.tile([P, C], f32, name=f"valT{k}") for k in range(KT)]
            with tc.tile_pool(name="setup", bufs=1) as spool:
                k1 = [spool.tile([P, K], f32, name=f"k1_{c}") for c in range(CT)]
                conv(p_t, 2, k1, K)
                conv(k1, 3, key, K)
                v1 = [spool.tile([P, K], f32, name=f"v1_{c}") for c in range(CT)]
                conv(p_t, 4, v1, K)
                val = [spool.tile([P, K], f32, name=f"val{c}") for c in range(CT)]
                conv(v1, 5, val, K)
                for k in range(KT):
                    pt = psa.tile([P, C], f32, name=f"ptv{k}", tag="aps")
                    for c in range(CT):
                        nc.tensor.matmul(
                            pt[:, c * P : (c + 1) * P].bitcast(f32r),
                            val[c][:, k * P : (k + 1) * P].bitcast(f32r),
                            idn[:].bitcast(f32r),
                            is_transpose=True,
                        )
                    nc.vector.tensor_copy(valT[k][:].bitcast(f32r), pt[:])

            # ---- main pipeline over pixel tiles ----
            def stage_a(t, xt=None):
                if xt is None:
                    xt = [
                        lpool.tile([P, T], f32, name=f"xt{c}", tag=f"xt{c}")
                        for c in range(CT)
                    ]
                    for c in range(CT):
                        nc.sync.dma_start(
                            xt[c][:].bitcast(f32r),
                            xin[c * P : (c + 1) * P, t * T : (t + 1) * T].bitcast(f32r),
                        )
                t1 = [lpool.tile([P, T], f32, name=f"t1_{c}", tag=f"t1{c}") for c in range(CT)]
                conv(xt, 0, t1, T)
                q = [lpool.tile([P, T], f32, name=f"q{c}", tag=f"q{c}") for c in range(CT)]
                conv(t1, 1, q, T)
                probT = [
                    lpool.tile([P, T], f32, name=f"pT{k}", tag=f"pT{k}")
                    for k in range(KT)
                ]
                for k in range(KT):
                    ps = psa.tile([P, T], f32, name=f"ps_simT{k}", tag="aps")
                    for c in range(CT):
                        nc.tensor.matmul(
                            ps[:],
                            key[c][:, k * P : (k + 1) * P].bitcast(f32r),
                            q[c][:].bitcast(f32r),
                            start=(c == 0),
                            stop=(c == CT - 1),
                        )
                    nc.scalar.activation(
                        probT[k][:].bitcast(f32r), ps[:], AF.Exp, scale=ALPHA
                    )
                return probT

            def stage_b(t, probT):
                ps_rs = psa.tile([P, T], f32, name="ps_rs", tag="aps")
                for k in range(KT):
                    nc.tensor.matmul(
                        ps_rs[:],
                        ones[:].bitcast(f32r),
                        probT[k][:].bitcast(f32r),
                        start=(k == 0),
                        stop=(k == KT - 1),
                    )
                rc = lpool.tile([P, T], f32, name="rc", tag="rc")
                nc.vector.reciprocal_approx_fast(out=rc[:], in_=ps_rs[:])
                ctx = [lpool.tile([P, T], f32, name=f"cx{c}", tag=f"cx{c}") for c in range(CT)]
                for c in range(CT):
                    ps = psa.tile([P, T], f32, name=f"ps_ctx{c}", tag="aps")
                    for k in range(KT):
                        nc.tensor.matmul(
                            ps[:],
                            valT[k][:, c * P : (c + 1) * P].bitcast(f32r),
                            probT[k][:].bitcast(f32r),
                            start=(k == 0),
                            stop=(k == KT - 1),
                        )
                    nc.vector.tensor_tensor(
                        out=ctx[c][:].bitcast(f32r),
                        in0=ps[:],
                        in1=rc[:],
                        op=mybir.AluOpType.mult,
                    )
                outt = [
                    lpool.tile([P, T], f32, name=f"ot{o}", tag=f"ot{o}") for o in range(CT)
                ]
                conv(ctx, 6, outt, T, out_f32r=False)
                for o in range(CT):
                    nc.sync.dma_start(
                        out_d[o * P : (o + 1) * P, t * T : (t + 1) * T], outt[o][:]
                    )

            prev = None
            for t in range(n_tiles):
                pT = stage_a(t, xt=xt0 if t == 0 else None)
                if prev is not None:
                    stage_b(prev[0], prev[1])
                prev = (t, pT)
            stage_b(prev[0], prev[1])

    nc.compile()
    return nc


def make_in_maps(x, proxy, W, s, b):
    # s > 0, so relu(s*(W@x)+b) == relu((diag(s)W)@x + b): fold s into W.
    w_eff = s[:, :, None].astype(np.float64) * W.astype(np.float64)
    wt = np.ascontiguousarray(w_eff.transpose(0, 2, 1)).astype(np.float32)
    sbc = np.ascontiguousarray(
        b.reshape(7, CT, P).transpose(2, 0, 1).reshape(P, 7 * CT)
    ).astype(np.float32)
    idn = np.eye(P, dtype=np.float32)
    ones = np.ones((P, P), dtype=np.float32)
    in_maps = []
    for n in range(N):
        in_maps.append(
            {
                "xin": np.ascontiguousarray(x[n].reshape(C, HW), dtype=np.float32),
                "pin": np.ascontiguousarray(proxy[n].reshape(C, K), dtype=np.float32),
                "wt": wt,
                "sbc": sbc,
                "idn": idn,
                "ones": ones,
            }
        )
    return in_maps


_CACHED = {}


def _get_module():
    if "nc" not in _CACHED:
        _CACHED["nc"] = build_module()
    return _CACHED["nc"]


def kernel(x, proxy, W, s, b):
    nc = _get_module()
    in_maps = make_in_maps(x, proxy, W, s, b)
    res = bass_utils.run_bass_kernel_spmd(nc, in_maps, core_ids=list(range(N)))
    out = np.stack([res.results[n]["out"].reshape(C, H, W_IMG) for n in range(N)])
    return out.astype(np.float32)
